# revision 1
# baseline (speedup 1.0000x reference)
"""DeformConv2D Trainium2 kernel v2: main loop as tc.For_i hardware loop.

Same algorithm as the unrolled baseline (SWDGE gather of bilinear corner
row-pairs, DVE blend with per-partition scalars, PE transpose, fp32r
matmul), but the 16 chunk-group iterations run as one hardware loop so
the emitted program is ~16x smaller. Output accumulates in SBUF and is
written with one DMA after the loop.
"""
import sys
import numpy as np

sys.path.insert(0, "/opt/trn_rl_repo")

KS, PAD = 3, 1
B, C, H, W = 8, 256, 64, 64
OUTC = 256
N = KS * KS
HP = H + 2 * PAD
MARG = 8
HE = HP + 2 * MARG
SE = HE * HE
NPIX = H * W
NS = N * NPIX
NF = NS // 128            # 288
NCHUNK = NPIX // 128      # 32
IDX_PER_CHUNK = 2 * N * 128
TBL_COLS = IDX_PER_CHUNK // 16       # 144

_BUILT = None


def _build(num_devices=8, gsplit=3):
    import concourse.bass as bass
    import concourse.bacc as bacc
    import concourse.mybir as mybir
    import concourse.tile as tile
    import concourse.masks as masks
    from concourse.bass import ds

    dt = mybir.dt
    alu = mybir.AluOpType

    nc = bacc.Bacc("TRN2", target_bir_lowering=False, debug=False,
                   num_devices=num_devices)

    i_xe = nc.dram_tensor("xe", [SE, C], dt.float32, kind="ExternalInput").ap()
    i_off = nc.dram_tensor("off", [128, 2 * NF], dt.float32, kind="ExternalInput").ap()
    i_grid = nc.dram_tensor("grid", [128, 2 * NF], dt.float32, kind="ExternalInput").ap()
    i_wt = nc.dram_tensor("wt", [2 * N, 128, OUTC], dt.float32, kind="ExternalInput").ap()
    o_out = nc.dram_tensor("out", [2, 128, NPIX], dt.float32, kind="ExternalOutput").ap()

    xe_view = bass.AP(i_xe.tensor, 0, [[C, SE - 1], [1, 2 * C]])

    with tile.TileContext(nc) as tc:
        with (
            tc.tile_pool(name="const", bufs=1) as cp,
            tc.tile_pool(name="scr", bufs=1) as sp,
            tc.tile_pool(name="gat", bufs=1) as gp,
            tc.tile_pool(name="xo", bufs=1) as xp_,
            tc.tile_pool(name="xoT", bufs=1) as tp,
            tc.tile_pool(name="osb", bufs=1) as op_,
            tc.tile_pool(name="pst", bufs=2, space="PSUM") as pst,
            tc.tile_pool(name="psm", bufs=2, space="PSUM") as psm,
        ):
            # ---------- constants / inputs ----------
            wt = cp.tile([128, 2 * N, OUTC], dt.float32r)
            with tc.tile_pool(name="tmpw", bufs=1) as twp:
                wt_f = twp.tile([128, 2 * N, OUTC], dt.float32)
                nc.sync.dma_start(wt_f[:], i_wt.transpose([1, 0, 2]))
                nc.vector.tensor_copy(wt[:], wt_f[:])

            ident_f = cp.tile([128, 128], dt.float32)
            masks.make_identity(nc, ident_f[:])
            ident = cp.tile([128, 128], dt.float32r)
            nc.vector.tensor_copy(ident[:], ident_f[:])

            off = cp.tile([128, 2 * NF], dt.float32)
            grid = cp.tile([128, 2 * NF], dt.float32)
            nc.sync.dma_start(off[:], i_off)
            nc.sync.dma_start(grid[:], i_grid)

            # ---------- offset math (identical to baseline) ----------
            p = sp.tile([128, 2 * NF], dt.float32, name="p", tag="sA")
            nc.vector.tensor_tensor(p[:], grid[:], off[:], alu.add)
            ri = sp.tile([128, 2 * NF], dt.int32, name="ri", tag="sB")
            nc.vector.tensor_copy(ri[:], p[:])
            rf = sp.tile([128, 2 * NF], dt.float32, name="rf", tag="sC")
            nc.vector.tensor_copy(rf[:], ri[:])
            gtv = sp.tile([128, 2 * NF], dt.float32, name="gtv", tag="sD")
            nc.vector.tensor_tensor(gtv[:], rf[:], p[:], alu.is_gt)
            fl = sp.tile([128, 2 * NF], dt.float32, name="fl", tag="sE")
            nc.vector.tensor_tensor(fl[:], rf[:], gtv[:], alu.subtract)
            t1 = sp.tile([128, 2 * NF], dt.float32, name="t1", tag="sB")
            nc.vector.tensor_scalar(t1[:], p[:], 1.0, None, alu.is_ge)
            t2 = sp.tile([128, 2 * NF], dt.float32, name="t2", tag="sC")
            nc.vector.tensor_scalar(t2[:], p[:], float(HP - 2), None, alu.is_le)
            nc.vector.tensor_tensor(t1[:], t1[:], t2[:], alu.mult)
            fr = sp.tile([128, 2 * NF], dt.float32, name="fr", tag="sD")
            nc.vector.tensor_tensor(fr[:], p[:], fl[:], alu.subtract)
            w1 = sp.tile([128, 2 * NF], dt.float32, name="w1", tag="sA")
            nc.vector.tensor_tensor(w1[:], fr[:], t1[:], alu.mult)
            w0 = sp.tile([128, 2 * NF], dt.float32, name="w0", tag="sC")
            nc.vector.tensor_scalar(w0[:], w1[:], -1.0, 1.0, alu.mult, alu.add)
            w00 = cp.tile([128, NF], dt.float32)
            w01 = cp.tile([128, NF], dt.float32)
            w10 = cp.tile([128, NF], dt.float32)
            w11 = cp.tile([128, NF], dt.float32)
            nc.vector.tensor_tensor(w00[:], w0[:, :NF], w0[:, NF:], alu.mult)
            nc.vector.tensor_tensor(w01[:], w0[:, :NF], w1[:, NF:], alu.mult)
            nc.vector.tensor_tensor(w10[:], w1[:, :NF], w0[:, NF:], alu.mult)
            nc.vector.tensor_tensor(w11[:], w1[:, :NF], w1[:, NF:], alu.mult)
            ic = sp.tile([128, 2 * NF], dt.float32, name="ic", tag="sB")
            nc.vector.tensor_scalar(ic[:], fl[:], float(HP - 2 + MARG),
                                    float(-MARG), alu.min, alu.max)
            idxf = sp.tile([128, NF], dt.float32, name="idxf", tag="sF")
            nc.vector.tensor_scalar(idxf[:], ic[:, :NF], float(HE),
                                    float(MARG * HE + MARG), alu.mult, alu.add)
            nc.vector.tensor_tensor(idxf[:], idxf[:], ic[:, NF:], alu.add)

            # ---------- fold to 16-part idx table ----------
            fold = sp.tile([16, NF * 8], dt.float32)
            for q in range(8):
                nc.sync.dma_start(
                    fold[:, q:NF * 8:8].unsqueeze(-1),
                    idxf[q * 16:(q + 1) * 16, :].unsqueeze(-1),
                )
            table = cp.tile([128, NCHUNK * TBL_COLS], dt.int16)
            t0_ap = bass.AP(table.tensor, table[:16].offset,
                            [table[:16].ap[0], [TBL_COLS, NCHUNK], [1, 72]])
            f_ap = bass.AP(fold.tensor, fold[:].offset,
                           [fold[:].ap[0], [72, NCHUNK], [1, 72]])
            nc.vector.tensor_copy(t0_ap, f_ap)
            t1_ap = bass.AP(table.tensor, table[:16].offset + 72,
                            [table[:16].ap[0], [TBL_COLS, NCHUNK], [1, 72]])
            nc.vector.tensor_scalar(t1_ap, f_ap, float(HE), None, alu.add)
            for rep in range(3):
                span = 16 << rep
                nc.sync.dma_start(table[span:2 * span, :], table[:span, :])

            # ---------- output accumulator in SBUF ----------
            obig = cp.tile([128, 2, NPIX], dt.float32)

            # ---------- tiles reused across loop iterations ----------
            g = gp.tile([128, 2 * N, 2 * C], dt.float32, name="g", tag="g")
            m0 = xp_.tile([128, N, 256], dt.float32r, name="m0", tag="m0")
            mt = xp_.tile([128, N, 256], dt.float32r, name="mt", tag="mt")
            # [c-half part, t, cb, half, pix128]
            xoTT = tp.tile([128, N, 2, 2, 128], dt.float32r,
                           name="xoTT", tag="xoTT")

            nidx = IDX_PER_CHUNK // gsplit
            tcols = TBL_COLS // gsplit
            ngrp = nidx // 128

            # ---------- main hardware loop over 16 chunk pairs ----------
            with tc.For_i(0, NCHUNK // 2) as sc:
                for half in range(2):
                    # ch = 2*sc + half
                    for gs in range(gsplit):
                        col0 = sc * (2 * TBL_COLS) + (half * TBL_COLS + gs * tcols)
                        nc.gpsimd.dma_gather(
                            g[:, gs * ngrp:(gs + 1) * ngrp, :], xe_view,
                            table[:, ds(col0, tcols)],
                            num_idxs=nidx, num_idxs_reg=nidx,
                            elem_size=2 * C, elem_step=C,
                            single_packet=True,
                        )
                    # blend all 9 taps at once: per-(partition, tap) scalars
                    # broadcast over the 256 channel columns (stride-0 axis)
                    wc0 = sc * (2 * N) + half * N
                    b00 = w00[:, ds(wc0, N)].unsqueeze(-1).broadcast_to(
                        [128, N, 256])
                    b01 = w01[:, ds(wc0, N)].unsqueeze(-1).broadcast_to(
                        [128, N, 256])
                    b10 = w10[:, ds(wc0, N)].unsqueeze(-1).broadcast_to(
                        [128, N, 256])
                    b11 = w11[:, ds(wc0, N)].unsqueeze(-1).broadcast_to(
                        [128, N, 256])
                    nc.vector.tensor_tensor(m0[:], g[:, 0:N, 0:256], b00,
                                            alu.mult)
                    nc.vector.tensor_tensor(mt[:], g[:, 0:N, 256:512], b01,
                                            alu.mult)
                    nc.vector.tensor_tensor(m0[:], m0[:], mt[:], alu.add)
                    nc.vector.tensor_tensor(mt[:], g[:, N:2 * N, 0:256], b10,
                                            alu.mult)
                    nc.vector.tensor_tensor(m0[:], m0[:], mt[:], alu.add)
                    nc.vector.tensor_tensor(mt[:], g[:, N:2 * N, 256:512], b11,
                                            alu.mult)
                    nc.vector.tensor_tensor(m0[:], m0[:], mt[:], alu.add)
                    # transpose 18 (t, cb) blocks; batch PSUM drains 4-at-a-time
                    for q in range(5):
                        nq = 4 if q < 4 else 2
                        ptr = pst.tile([128, 512], dt.float32r, tag="ptr")
                        for j in range(nq):
                            kt = 4 * q + j
                            t, cb = kt // 2, kt % 2
                            nc.tensor.transpose(
                                ptr[:, j * 128:(j + 1) * 128],
                                m0[:, t, cb * 128:(cb + 1) * 128],
                                ident[:])
                        t0 = (4 * q) // 2
                        dst = bass.AP(
                            xoTT.tensor,
                            xoTT[:].offset + t0 * 512 + half * 128,
                            [xoTT[:].ap[0], [512, nq // 2], [256, 2], [1, 128]])
                        nc.scalar.copy(dst, ptr[:, :nq * 128])
                for hf in range(2):
                    pm = psm.tile([128, 256], dt.float32, tag="pm")
                    for kt in range(2 * N):
                        t, cb = kt // 2, kt % 2
                        nc.tensor.matmul(
                            pm[:], wt[:, kt, hf * 128:(hf + 1) * 128],
                            xoTT[:, t, cb, :, :],
                            start=(kt == 0), stop=(kt == 2 * N - 1))
                    nc.scalar.copy(obig[:, hf, ds(sc * 256, 256)], pm[:])

            # ---------- single output DMA ----------
            nc.sync.dma_start(o_out.transpose([1, 0, 2]), obig[:])

    nc.compile()
    return nc


def _host_prep(x, offset, weight):
    xe = np.zeros((B, HE, HE, C), dtype=np.float32)
    xe[:, MARG + 1:MARG + 1 + H, MARG + 1:MARG + 1 + W, :] = \
        x.transpose(0, 2, 3, 1)
    xe = xe.reshape(B, SE, C)

    def lay(o):
        o = o.reshape(B, N, NPIX // 128, 128)
        return np.ascontiguousarray(
            o.transpose(0, 2, 1, 3).reshape(B, NF, 128).transpose(0, 2, 1))

    ox = lay(offset[:, 0::2])
    oy = lay(offset[:, 1::2])
    off = np.concatenate([ox, oy], axis=2)

    r = np.arange(-(KS - 1) // 2, (KS - 1) // 2 + 1)
    pnx, pny = np.meshgrid(r, r, indexing="ij")
    i_idx, j_idx = np.meshgrid(np.arange(1, H + 1), np.arange(1, W + 1),
                               indexing="ij")
    gx = (i_idx.reshape(-1).astype(np.float32).reshape(NPIX // 128, 1, 128)
          + pnx.reshape(-1).astype(np.float32).reshape(1, N, 1))
    gy = (j_idx.reshape(-1).astype(np.float32).reshape(NPIX // 128, 1, 128)
          + pny.reshape(-1).astype(np.float32).reshape(1, N, 1))
    gx = gx.reshape(NF, 128).T
    gy = gy.reshape(NF, 128).T
    grid = np.ascontiguousarray(np.concatenate([gx, gy], axis=1))

    wt = weight.reshape(OUTC, C, N).transpose(2, 1, 0)
    wt = np.ascontiguousarray(
        wt.reshape(N, 2, 128, OUTC).reshape(2 * N, 128, OUTC))
    return xe, off, grid, wt


def kernel(x, offset, weight):
    global _BUILT
    from concourse.bass_utils import run_bass_kernel_spmd

    x = np.asarray(x, dtype=np.float32)
    offset = np.asarray(offset, dtype=np.float32)
    weight = np.asarray(weight, dtype=np.float32)

    xe, off, grid, wt = _host_prep(x, offset, weight)
    if _BUILT is None:
        _BUILT = _build()
    nc = _BUILT

    in_maps = [
        {"xe": xe[b], "off": off[b], "grid": grid, "wt": wt}
        for b in range(B)
    ]
    res = run_bass_kernel_spmd(nc, in_maps, list(range(B)))
    out = np.stack([
        res.results[b]["out"].reshape(OUTC, H, W) for b in range(B)
    ])
    return out



# revision 2
# speedup vs baseline: 303.2532x; 303.2532x over previous
"""DeformConv2D Trainium2 kernel v3.

Changes vs v2 baseline:
- Host precomputes gather indices + bilinear corner weights (numpy).
- xe packed as row-pairs in bf16: xe2[s] = [x[s], x[s+HE]] channels, so ONE
  2KB descriptor fetches all 4 bilinear corners of a sample (descriptor
  count halved to 36864/core, bytes halved to ~74MB/core).
- bf16 datapath: gather, blend (DVE 2x), PE transpose + matmul (bf16).
- Fully unrolled straight-line program (32 chunks, no For_i barriers) with
  double/triple-buffered tiles so SWDGE gather (bottleneck) overlaps DVE
  blend, PE transpose/matmul and ACT PSUM drains.
"""
import sys
import numpy as np

sys.path.insert(0, "/opt/trn_rl_repo")

import ml_dtypes

BF16 = ml_dtypes.bfloat16

KS, PAD = 3, 1
B, C, H, W = 8, 256, 64, 64
OUTC = 256
N = KS * KS                  # 9 taps
HP = H + 2 * PAD             # 66
MARG = 8
HE = HP + 2 * MARG           # 82
SE = HE * HE                 # 6724
NPIX = H * W                 # 4096
NCHUNK = NPIX // 128         # 32
NF = NCHUNK * N              # 288
TCOLS = N * 128 // 16        # 72 idx-table cols per chunk

_BUILT = None


def _build(num_devices=8, gbufs=3, reps=1, nq=1, gsplit=3):
    import concourse.bass as bass
    import concourse.bacc as bacc
    import concourse.mybir as mybir
    import concourse.tile as tile
    import concourse.masks as masks
    from concourse.bass import ds

    dt = mybir.dt
    alu = mybir.AluOpType

    nc = bacc.Bacc("TRN2", target_bir_lowering=False, debug=False,
                   num_devices=num_devices, num_swdge_queues=nq)

    i_xe = nc.dram_tensor("xe2", [SE, 2 * C], dt.bfloat16,
                          kind="ExternalInput").ap()
    i_tbl = nc.dram_tensor("tbl", [128, NCHUNK * TCOLS], dt.int16,
                           kind="ExternalInput").ap()
    i_wq = nc.dram_tensor("wq", [128, 4, NF], dt.bfloat16,
                          kind="ExternalInput").ap()
    i_wt = nc.dram_tensor("wt", [2 * N, 128, OUTC], dt.bfloat16,
                          kind="ExternalInput").ap()
    o_out = nc.dram_tensor("out", [2, 128, NPIX], dt.float32,
                           kind="ExternalOutput").ap()

    # gather source view: per-index stride 2C elems, elem covers 2 pixels
    xe_view = bass.AP(i_xe.tensor, 0, [[2 * C, SE - 1], [1, 4 * C]])

    with tile.TileContext(nc) as tc:
        with (
            tc.tile_pool(name="const", bufs=1) as cp,
            tc.tile_pool(name="gat", bufs=gbufs) as gp,
            tc.tile_pool(name="m", bufs=2) as mp,
            tc.tile_pool(name="xoT", bufs=2) as tp,
            tc.tile_pool(name="pst", bufs=4, space="PSUM") as pst,
            tc.tile_pool(name="psm", bufs=2, space="PSUM") as psm,
        ):
            # ---------- constants ----------
            wt = cp.tile([128, 2 * N, OUTC], dt.bfloat16)
            nc.sync.dma_start(wt[:], i_wt.transpose([1, 0, 2]))

            ident_f = cp.tile([128, 128], dt.float32)
            masks.make_identity(nc, ident_f[:])
            ident = cp.tile([128, 128], dt.bfloat16)
            nc.vector.tensor_copy(ident[:], ident_f[:])

            table = cp.tile([128, NCHUNK * TCOLS], dt.int16)
            nc.sync.dma_start(table[:], i_tbl)
            wq = cp.tile([128, 4, NF], dt.bfloat16)
            nc.sync.dma_start(wq[:], i_wq)

            obig = cp.tile([128, 2, NPIX], dt.float32)

            # ---------- main: 32 chunks of 128 pixels ----------
            import contextlib
            rctx = tc.For_i(0, reps) if reps > 1 else contextlib.nullcontext()
            with rctx:
             for ch in range(NCHUNK):
                g = gp.tile([128, N, 4 * C], dt.bfloat16, tag="g")
                ntap = N // gsplit
                nidx = ntap * 128
                tc_g = TCOLS // gsplit
                for gs in range(gsplit):
                    nc.gpsimd.dma_gather(
                        g[:, gs * ntap:(gs + 1) * ntap, :], xe_view,
                        table[:, ds(ch * TCOLS + gs * tc_g, tc_g)],
                        num_idxs=nidx, num_idxs_reg=nidx,
                        elem_size=4 * C, elem_step=2 * C,
                        single_packet=True,
                        queue_num=(gs % nq),
                    )
                # blend 4 corners with per-(pixel,tap) weights
                m0 = mp.tile([128, N, 256], dt.bfloat16, tag="m0")
                mt = mp.tile([128, N, 256], dt.bfloat16, tag="mt")
                wv = [wq[:, q, ds(ch * N, N)].unsqueeze(-1).broadcast_to(
                    [128, N, 256]) for q in range(4)]
                nc.vector.tensor_tensor(m0[:], g[:, :, 0:256], wv[0], alu.mult)
                nc.vector.tensor_tensor(mt[:], g[:, :, 256:512], wv[1],
                                        alu.mult)
                nc.vector.tensor_tensor(m0[:], m0[:], mt[:], alu.add)
                nc.vector.tensor_tensor(mt[:], g[:, :, 512:768], wv[2],
                                        alu.mult)
                nc.vector.tensor_tensor(m0[:], m0[:], mt[:], alu.add)
                nc.vector.tensor_tensor(mt[:], g[:, :, 768:1024], wv[3],
                                        alu.mult)
                nc.vector.tensor_tensor(m0[:], m0[:], mt[:], alu.add)

                # transpose [pix, c] -> [c, pix] per (tap, c-half): 18 blocks
                xoT = tp.tile([128, 2 * N, 128], dt.bfloat16, tag="xoT")
                for q in range(5):
                    nq = 4 if q < 4 else 2
                    ptr = pst.tile([128, 512], dt.float32, tag="ptr")
                    for j in range(nq):
                        kt = 4 * q + j
                        t, cb = kt // 2, kt % 2
                        nc.tensor.transpose(
                            ptr[:, j * 128:(j + 1) * 128],
                            m0[:, t, cb * 128:(cb + 1) * 128],
                            ident[:])
                    nc.scalar.copy(xoT[:, 4 * q:4 * q + nq, :],
                                   ptr[:, :nq * 128])

                # conv: accumulate 18 (tap, c-half) matmuls per outc-half
                for hf in range(2):
                    pm = psm.tile([128, 128], dt.float32, tag="pm")
                    for kt in range(2 * N):
                        nc.tensor.matmul(
                            pm[:], wt[:, kt, hf * 128:(hf + 1) * 128],
                            xoT[:, kt, :],
                            start=(kt == 0), stop=(kt == 2 * N - 1))
                    nc.scalar.copy(obig[:, hf, ds(ch * 128, 128)], pm[:])

            nc.sync.dma_start(o_out.transpose([1, 0, 2]), obig[:])

    nc.compile()
    return nc


def _host_prep(x, offset, weight):
    """Numpy: pack image, compute gather indices + bilinear weights."""
    x = np.asarray(x, np.float32)
    offset = np.asarray(offset, np.float32)
    weight = np.asarray(weight, np.float32)

    # xe2[b, s=(r,col), :] = [channels of (r,col), channels of (r+1,col)]
    xpadm = np.zeros((B, HE + 1, HE, C), dtype=np.float32)
    xpadm[:, MARG + 1:MARG + 1 + H, MARG + 1:MARG + 1 + W, :] = \
        x.transpose(0, 2, 3, 1)
    xe2 = np.concatenate([xpadm[:, :HE], xpadm[:, 1:HE + 1]], axis=3)
    xe2 = np.ascontiguousarray(xe2.reshape(B, SE, 2 * C)).astype(BF16)

    # sample positions p = base grid + tap offset + data offset (padded coords)
    off = offset.reshape(B, N, 2, H, W)
    ox, oy = off[:, :, 0], off[:, :, 1]                      # (B,N,H,W)
    r = np.arange(-(KS - 1) // 2, (KS - 1) // 2 + 1)
    pnx, pny = np.meshgrid(r, r, indexing="ij")
    gi = np.arange(1, H + 1).reshape(1, 1, H, 1)
    gj = np.arange(1, W + 1).reshape(1, 1, 1, W)
    px = gi + pnx.reshape(1, N, 1, 1) + ox                   # (B,N,H,W)
    py = gj + pny.reshape(1, N, 1, 1) + oy

    def comp(p):
        fl = np.floor(p)
        inb = ((p >= 1) & (p <= HP - 2)).astype(np.float32)
        w1 = (p - fl) * inb                                  # frac (0 at edge)
        ic = np.clip(fl, -MARG, HP - 2 + MARG)
        return w1.astype(np.float32), ic.astype(np.int32)

    w1x, icx = comp(px)
    w1y, icy = comp(py)
    idx = (icx + MARG) * HE + (icy + MARG)                   # (B,N,H,W) int32

    w0x, w0y = 1.0 - w1x, 1.0 - w1y
    # quarter order matches xe2 elem: [ (r,c), (r+1,c), (r,c+1), (r+1,c+1) ]
    wqs = np.stack([w0x * w0y, w1x * w0y, w0x * w1y, w1x * w1y], axis=1)

    # layouts: chunk ch = pixels [128ch,128(ch+1)), partition p = pixel in chunk
    idx = idx.reshape(B, N, NCHUNK, 128)
    tbl = idx.transpose(0, 2, 1, 3).reshape(B, NCHUNK, N, 8, 16)
    tbl = tbl.transpose(0, 4, 1, 2, 3).reshape(B, 16, NCHUNK * TCOLS)
    tbl = np.ascontiguousarray(np.tile(tbl, (1, 8, 1))).astype(np.int16)

    wqs = wqs.reshape(B, 4, N, NCHUNK, 128)
    wq = np.ascontiguousarray(
        wqs.transpose(0, 4, 1, 3, 2).reshape(B, 128, 4, NF)).astype(BF16)

    wt = weight.reshape(OUTC, C, N).transpose(2, 1, 0)
    wt = np.ascontiguousarray(
        wt.reshape(N, 2, 128, OUTC).reshape(2 * N, 128, OUTC)).astype(BF16)

    return [{"xe2": xe2[b], "tbl": tbl[b], "wq": wq[b], "wt": wt}
            for b in range(B)]


def kernel(x, offset, weight):
    global _BUILT
    from concourse.bass_utils import run_bass_kernel_spmd

    in_maps = _host_prep(x, offset, weight)
    if _BUILT is None:
        _BUILT = _build()
    res = run_bass_kernel_spmd(_BUILT, in_maps, list(range(B)))
    out = np.stack([
        res.results[b]["out"].reshape(OUTC, H, W) for b in range(B)
    ])
    return out
